# revision 4
# baseline (speedup 1.0000x reference)
"""Trainium2 Bass kernel for LowRankRayTracer.

csi[f] = (delta_t/D) * v_f^T M v_f,  M = conj(rad)^T conj(att)  (R=32, complex)
contracted over N = D*K = 524288 rows.

Strategy (8 cores):
  - Shard the N rows across cores (512 directions each): each core computes
    its partial S = rad^T att (128x128 f32; complex pairs via the f32 view +
    2-rows-per-partition packing).
  - AllReduce S across the 8 cores (64 KB, HBM bounce) so every core holds
    the full S; each core then builds W = [W_real|W_imag] and computes csi
    for ITS F/8 = 1024 subcarrier shard only. Host concatenates shards.
  - Precision budget: harness gate is rel_err < 2e-2, fp16-quantized inputs
    give ~5e-4, so rad/att/g/W/e are all fp16 "hi" only (no lo-correction
    passes): half the HBM bytes and a quarter of the PE columns vs the
    hi/lo-exact version.
  - Macro loads are split into multiple dma_starts to spread bytes evenly
    over the 16 DMA queues (a single dma_start lands on one ~22 GB/s queue).
  - Matmuls accumulate round-robin into 4 bank-sized PSUM tiles (avoids
    same-bank RMW serialization).
"""

import numpy as np

D, K, R = 4096, 128, 32
F = 8192
N_CORES = 8
DIR_PER_CORE = D // N_CORES              # 512
N_MACRO = 8                              # macro tiles per tensor per core
MACRO_COLS = 4096                        # fp16 per partition per macro tile
SLICE = 128                              # matmul slice width (2 rows/partition)
SCALE = (200.0 / K) / D                  # delta_t / num_directions (exact binary)
FSH = F // N_CORES                       # 1024 subcarriers per core
FCHUNK = 512                             # phase-3 subcarriers per chunk
N_FCHUNK = FSH // FCHUNK                 # 2
NB = 4                                   # round-robin PSUM accumulator banks

_NC_CACHE = {}


def _build_consts():
    """(128, 258) f32: four (128,64) selection matrices + ones-selector cols."""
    c = np.zeros((128, 258), np.float32)
    EA = np.zeros((128, 32), np.float32)
    OA = np.zeros((128, 32), np.float32)
    EB = np.zeros((128, 32), np.float32)
    OB = np.zeros((128, 32), np.float32)
    for m in range(32):
        EA[2 * m, m] = 1.0
        OA[2 * m + 1, m] = 1.0
        EB[64 + 2 * m, m] = 1.0
        OB[64 + 2 * m + 1, m] = 1.0
    c[:, 0:32] = EA
    c[:, 32:64] = OA
    c[:, 64:96] = EB
    c[:, 96:128] = OB
    c[:, 128:160] = OA
    c[:, 160:192] = EA
    c[:, 192:224] = OB
    c[:, 224:256] = EB
    c[0:64, 256] = 1.0
    c[64:128, 257] = 1.0
    return c


def build_nc(n_macro=N_MACRO):
    import concourse.bacc as bacc
    import concourse.mybir as mybir
    import concourse.tile as tile

    fp32 = mybir.dt.float32
    fp16 = mybir.dt.float16
    nc = bacc.Bacc(trn_type="TRN2", target_bir_lowering=False, debug=False)

    rad_d = nc.dram_tensor("rad_h", [n_macro, 128, MACRO_COLS], fp16,
                           kind="ExternalInput").ap()
    att_d = nc.dram_tensor("att_h", [n_macro, 128, MACRO_COLS], fp16,
                           kind="ExternalInput").ap()
    gth_d = nc.dram_tensor("gth", [64, FSH], fp16, kind="ExternalInput").ap()
    cst_d = nc.dram_tensor("consts", [128, 258], fp32, kind="ExternalInput").ap()
    out_d = nc.dram_tensor("csi", [2, FSH], fp32, kind="ExternalOutput").ap()

    with tile.TileContext(nc) as tc:
        with (
            tc.tile_pool(name="io", bufs=2) as io_pool,
            tc.tile_pool(name="small", bufs=1) as small,
            tc.tile_pool(name="epool", bufs=4) as epool,
            tc.tile_pool(name="dram", bufs=1, space="DRAM") as dram,
        ):
            # constants up front (tiny); g loads issued after the first macro
            c_sb = small.tile([128, 258], fp32, tag="consts")
            nc.sync.dma_start(c_sb[:], cst_d[:])
            g2_sb = small.tile([128, FSH], fp16, tag="g2")

            # ---- main loop: S += rad^T att, fp16 hi-only ----
            s_sb = small.tile([128, 128], fp32, tag="s_sb")
            n_slices = MACRO_COLS // SLICE
            total = n_macro * n_slices
            with tc.tile_pool(name="spsum", bufs=1, space="PSUM") as spsum:
                banks = [spsum.tile([128, 512], fp32, tag=f"s{b}",
                                    name=f"sbank{b}")
                         for b in range(NB)]
                seen = [False] * NB
                idx = 0
                for i in range(n_macro):
                    rad = io_pool.tile([128, MACRO_COLS], fp16, tag="rad")
                    att = io_pool.tile([128, MACRO_COLS], fp16, tag="att")
                    # chunked loads: spread bytes over many DMA queues, and
                    # let the first matmuls start after ~0.5 MiB, not 2 MiB
                    nch = 4 if i == 0 else 2
                    cm = MACRO_COLS // nch
                    for q in range(nch):
                        qs = slice(q * cm, (q + 1) * cm)
                        nc.sync.dma_start(rad[:, qs], rad_d[i, :, qs])
                        nc.scalar.dma_start(att[:, qs], att_d[i, :, qs])
                    if i == 0:
                        # duplicated g shard (fp16, 128 partitions): two HBM
                        # reads of the same [64, FSH] tensor (tiny)
                        nc.sync.dma_start(g2_sb[0:64, :], gth_d[:])
                        nc.sync.dma_start(g2_sb[64:128, :], gth_d[:])
                    for s in range(n_slices):
                        sl = slice(s * SLICE, (s + 1) * SLICE)
                        b = idx % NB
                        nc.tensor.matmul(
                            banks[b][:, 0:SLICE],
                            lhsT=rad[:, sl],
                            rhs=att[:, sl],
                            start=not seen[b],
                            stop=(idx >= total - NB),
                        )
                        seen[b] = True
                        idx += 1

                # S = sum of the 4 round-robin banks (DVE may read at most
                # one PSUM operand per instruction)
                nc.vector.tensor_copy(s_sb[:], banks[0][:, 0:SLICE])
                for b in range(1, NB):
                    nc.vector.tensor_add(s_sb[:], s_sb[:], banks[b][:, 0:SLICE])

            # ---- AllReduce S across the 8 cores (HBM bounce buffers) ----
            s_in = dram.tile([128, 128], fp32, tag="s_in")
            s_out = dram.tile([128, 128], fp32, tag="s_out")
            nc.sync.dma_start(s_in[:], s_sb[:])
            nc.gpsimd.collective_compute(
                "AllReduce",
                mybir.AluOpType.add,
                replica_groups=[list(range(N_CORES))],
                ins=[s_in[:]],
                outs=[s_out[:]],
            )
            sr_sb = small.tile([128, 128], fp32, tag="sr_sb")
            nc.sync.dma_start(sr_sb[:], s_out[:])

            # PE warm-keepers: cheap matmuls dependent on s_sb bridge the
            # collective+epilogue gap so HAM doesn't re-throttle
            with tc.tile_pool(name="wpsum", bufs=1, space="PSUM") as wpsum:
                warm_ps = wpsum.tile([64, 64], fp32, tag="warm")
                for w in range(10):
                    nc.tensor.matmul(warm_ps[:], lhsT=c_sb[:, 0:64],
                                     rhs=s_sb[:, 0:64], start=True, stop=True)

            # ---- epilogue: build W = [W_real | W_imag] (64, 128) ----
            with tc.tile_pool(name="vpsum", bufs=1, space="PSUM") as vpsum:
                v1 = vpsum.tile([64, 64], fp32, tag="v1")
                nc.tensor.matmul(v1[:], lhsT=c_sb[:, 0:64], rhs=sr_sb[:, 0:64],
                                 start=True, stop=False)
                nc.tensor.matmul(v1[:], lhsT=c_sb[:, 64:128],
                                 rhs=sr_sb[:, 64:128], start=False, stop=True)
                v2 = vpsum.tile([64, 64], fp32, tag="v2")
                nc.tensor.matmul(v2[:], lhsT=c_sb[:, 128:192],
                                 rhs=sr_sb[:, 0:64], start=True, stop=False)
                nc.tensor.matmul(v2[:], lhsT=c_sb[:, 192:256],
                                 rhs=sr_sb[:, 64:128], start=False, stop=True)

                v1s = small.tile([64, 64], fp32, tag="v1s")
                nc.vector.tensor_copy(v1s[:], v1[:])
                v2s = small.tile([64, 64], fp32, tag="v2s")
                nc.vector.tensor_copy(v2s[:], v2[:])

            # mr = Mr (dup-stacked), mp = -Mi (dup-stacked)
            mr = small.tile([64, 32], fp32, tag="mr")
            mp = small.tile([64, 32], fp32, tag="mp")
            nc.vector.tensor_sub(mr[0:32, :], v1s[0:32, 0:64:2], v2s[0:32, 1:64:2])
            nc.vector.tensor_sub(mr[32:64, :], v2s[32:64, 0:64:2], v1s[32:64, 1:64:2])
            nc.vector.tensor_add(mp[0:32, :], v1s[0:32, 1:64:2], v2s[0:32, 0:64:2])
            nc.vector.tensor_add(mp[32:64, :], v2s[32:64, 1:64:2], v1s[32:64, 0:64:2])

            wri = small.tile([64, 128], fp32, tag="wri")
            s_ = float(SCALE)
            # W_real = [[Mr, -Mi], [-Mi, -Mr]] * s
            nc.scalar.mul(wri[0:32, 0:32], mr[0:32, :], s_)
            nc.scalar.mul(wri[0:32, 32:64], mp[0:32, :], s_)
            nc.scalar.mul(wri[32:64, 0:32], mp[32:64, :], s_)
            nc.scalar.mul(wri[32:64, 32:64], mr[32:64, :], -s_)
            # W_imag = [[Mi, Mr], [Mr, -Mi]] * s
            nc.scalar.mul(wri[0:32, 64:96], mp[0:32, :], -s_)
            nc.scalar.mul(wri[0:32, 96:128], mr[0:32, :], s_)
            nc.scalar.mul(wri[32:64, 64:96], mr[32:64, :], s_)
            nc.scalar.mul(wri[32:64, 96:128], mp[32:64, :], s_)

            # fp16 W for the phase-3 matmuls; fp16 ones-selector for the
            # csi reduction (matmul operands must both be non-fp32)
            wh = small.tile([64, 128], fp16, tag="wh")
            nc.vector.tensor_copy(wh[:], wri[:])
            sel16 = small.tile([128, 2], fp16, tag="sel16")
            nc.vector.tensor_copy(sel16[:], c_sb[:, 256:258])

            # ---- phase 3: csi over this core's F shard ----
            csi_sb = small.tile([2, FSH], fp32, tag="csi_sb")
            with (
                tc.tile_pool(name="tpsum", bufs=2, space="PSUM") as tpsum,
                tc.tile_pool(name="cpsum", bufs=2, space="PSUM") as cpsum,
            ):
                for ci in range(N_FCHUNK):
                    fs = slice(ci * FCHUNK, (ci + 1) * FCHUNK)
                    t_ps = tpsum.tile([128, FCHUNK], fp32, tag="t",
                                      name=f"t{ci}")
                    # T = W^T g, fp16 single pass
                    nc.tensor.matmul(t_ps[:], lhsT=wh[:], rhs=g2_sb[0:64, fs],
                                     start=True, stop=True)
                    e_sb = epool.tile([128, FCHUNK], fp16, tag="e",
                                      name=f"e{ci}")
                    nc.vector.tensor_mul(e_sb[:], g2_sb[:, fs], t_ps[:])
                    c_ps = cpsum.tile([2, FCHUNK], fp32, tag="c",
                                      name=f"c{ci}")
                    nc.tensor.matmul(c_ps[:], lhsT=sel16[:], rhs=e_sb[:],
                                     start=True, stop=True)
                    nc.scalar.copy(csi_sb[:, fs], c_ps[:])

            nc.sync.dma_start(out_d[:], csi_sb[:])

    nc.compile()
    return nc


def _prep_shared(fbv):
    """gth (64, F) fp16 from complex fbv (F, R): rows = [Re ranks; Im ranks]."""
    fbv32 = np.ascontiguousarray(fbv).view(np.float32).reshape(F, 2 * R)
    gbt = np.ascontiguousarray(
        np.concatenate([fbv32[:, 0::2].T, fbv32[:, 1::2].T], axis=0))
    return gbt.astype(np.float16)


def _shard_h(arr, core):
    """Core's complex64 shard -> fp16 hi array (N_MACRO, 128, MACRO_COLS)."""
    sh = arr[core * DIR_PER_CORE:(core + 1) * DIR_PER_CORE]
    f32 = np.ascontiguousarray(sh).view(np.float32).ravel()
    return f32.astype(np.float16).reshape(N_MACRO, 128, MACRO_COLS)


def kernel(attenuation_vectors, radiation_vectors, frequency_basis_vectors):
    from concourse.bass_utils import run_bass_kernel_spmd

    if "nc" not in _NC_CACHE:
        _NC_CACHE["nc"] = build_nc()
    nc = _NC_CACHE["nc"]

    gth = _prep_shared(frequency_basis_vectors)
    consts = _build_consts()
    in_maps = []
    for c in range(N_CORES):
        in_maps.append({
            "rad_h": _shard_h(radiation_vectors, c),
            "att_h": _shard_h(attenuation_vectors, c),
            "gth": np.ascontiguousarray(gth[:, c * FSH:(c + 1) * FSH]),
            "consts": consts,
        })

    res = run_bass_kernel_spmd(nc, in_maps, core_ids=list(range(N_CORES)))
    csi = np.zeros((2, F), np.float32)
    for c, r in enumerate(res.results):
        csi[:, c * FSH:(c + 1) * FSH] = r["csi"]
    return (csi[0] + 1j * csi[1]).astype(np.complex64)


# revision 7
# speedup vs baseline: 1.2808x; 1.2808x over previous
"""Trainium2 Bass kernel for LowRankRayTracer.

csi[f] = (delta_t/D) * v_f^T M v_f,  M = conj(rad)^T conj(att)  (R=32, complex)
contracted over N = D*K = 524288 rows.

Strategy (8 cores):
  - Shard the N rows across cores (512 directions each). csi is linear in M,
    so each core computes its partial S = rad^T att (128x128 f32; complex
    pairs via the f32 view + 2-rows-per-partition packing), builds
    W = [W_real|W_imag], computes partial csi over ALL F=8192 subcarriers,
    and the host sums the 8 partial csi vectors.
  - Precision budget: harness gate is rel_err < 2e-2, fp16-quantized inputs
    give ~5e-4, so rad/att/g/W/e are all fp16 "hi" only (no lo-correction
    passes): half the HBM bytes and a quarter of the PE columns vs the
    hi/lo-exact version.
  - Every load is split into multiple dma_starts to spread bytes evenly over
    the 16 DMA queues (a single dma_start lands on one ~22 GB/s queue).
  - Matmuls accumulate round-robin into 4 bank-sized PSUM tiles (avoids
    same-bank RMW serialization); phase 3 is software-pipelined so the PE
    never waits on the vector engine between chunks.
"""

import numpy as np

D, K, R = 4096, 128, 32
F = 8192
N_CORES = 8
DIR_PER_CORE = D // N_CORES              # 512
N_MACRO = 8                              # macro tiles per tensor per core
MACRO_COLS = 4096                        # fp16 per partition per macro tile
SLICE = 128                              # matmul slice width (2 rows/partition)
SCALE = (200.0 / K) / D                  # delta_t / num_directions (exact binary)
FCHUNK = 512                             # phase-3 subcarriers per chunk
                                         # (matmul PSUM out max = 1 bank)
N_FCHUNK = F // FCHUNK                   # 16
NB = 4                                   # round-robin PSUM accumulator banks

_NC_CACHE = {}


def _build_consts():
    """(128, 258) f32: four (128,64) selection matrices + ones-selector cols."""
    c = np.zeros((128, 258), np.float32)
    EA = np.zeros((128, 32), np.float32)
    OA = np.zeros((128, 32), np.float32)
    EB = np.zeros((128, 32), np.float32)
    OB = np.zeros((128, 32), np.float32)
    for m in range(32):
        EA[2 * m, m] = 1.0
        OA[2 * m + 1, m] = 1.0
        EB[64 + 2 * m, m] = 1.0
        OB[64 + 2 * m + 1, m] = 1.0
    c[:, 0:32] = EA
    c[:, 32:64] = OA
    c[:, 64:96] = EB
    c[:, 96:128] = OB
    c[:, 128:160] = OA
    c[:, 160:192] = EA
    c[:, 192:224] = OB
    c[:, 224:256] = EB
    c[0:64, 256] = 1.0
    c[64:128, 257] = 1.0
    return c


def build_nc(n_macro=N_MACRO):
    import concourse.bacc as bacc
    import concourse.mybir as mybir
    import concourse.tile as tile

    fp32 = mybir.dt.float32
    fp16 = mybir.dt.float16
    nc = bacc.Bacc(trn_type="TRN2", target_bir_lowering=False, debug=False)

    rad_d = nc.dram_tensor("rad_h", [n_macro, 128, MACRO_COLS], fp16,
                           kind="ExternalInput").ap()
    att_d = nc.dram_tensor("att_h", [n_macro, 128, MACRO_COLS], fp16,
                           kind="ExternalInput").ap()
    gth_d = nc.dram_tensor("gth", [64, F], fp16, kind="ExternalInput").ap()
    cst_d = nc.dram_tensor("consts", [128, 258], fp32, kind="ExternalInput").ap()
    out_d = nc.dram_tensor("csi", [2, F], fp32, kind="ExternalOutput").ap()

    with tile.TileContext(nc) as tc:
        with (
            tc.tile_pool(name="io", bufs=2) as io_pool,
            tc.tile_pool(name="small", bufs=1) as small,
            tc.tile_pool(name="epool", bufs=4) as epool,
        ):
            # constants up front (tiny); g loads interleaved after early macros
            c_sb = small.tile([128, 258], fp32, tag="consts")
            nc.sync.dma_start(c_sb[:], cst_d[:])
            g2_sb = small.tile([128, F], fp16, tag="g2")

            # ---- main loop: S += rad^T att, fp16 hi-only ----
            s_sb = small.tile([128, 128], fp32, tag="s_sb")
            n_slices = MACRO_COLS // SLICE
            total = n_macro * n_slices
            with tc.tile_pool(name="spsum", bufs=1, space="PSUM") as spsum:
                banks = [spsum.tile([128, 512], fp32, tag=f"s{b}",
                                    name=f"sbank{b}")
                         for b in range(NB)]
                seen = [False] * NB
                idx = 0
                for i in range(n_macro):
                    rad = io_pool.tile([128, MACRO_COLS], fp16, tag="rad")
                    att = io_pool.tile([128, MACRO_COLS], fp16, tag="att")
                    # chunked loads: spread bytes over many DMA queues, and
                    # let the first matmuls start after ~0.5 MiB, not 2 MiB
                    nch = 4 if i == 0 else 2
                    cm = MACRO_COLS // nch
                    for q in range(nch):
                        qs = slice(q * cm, (q + 1) * cm)
                        nc.sync.dma_start(rad[:, qs], rad_d[i, :, qs])
                        nc.scalar.dma_start(att[:, qs], att_d[i, :, qs])
                    if i in (1, 2):
                        # duplicated g (fp16, 128 partitions): the [64, F]
                        # tensor read twice from HBM, 4 chunked dma_starts
                        # per half, interleaved mid-loop so they neither
                        # steal first-macro bandwidth nor gate phase 3
                        half = (0, 64) if i == 1 else (64, 128)
                        for q in range(4):
                            fs = slice(q * (F // 4), (q + 1) * (F // 4))
                            nc.sync.dma_start(
                                g2_sb[half[0]:half[1], fs], gth_d[:, fs])
                    for s in range(n_slices):
                        sl = slice(s * SLICE, (s + 1) * SLICE)
                        b = idx % NB
                        nc.tensor.matmul(
                            banks[b][:, 0:SLICE],
                            lhsT=rad[:, sl],
                            rhs=att[:, sl],
                            start=not seen[b],
                            stop=(idx >= total - NB),
                        )
                        seen[b] = True
                        idx += 1

                # S = sum of the 4 round-robin banks (DVE may read at most
                # one PSUM operand per instruction)
                nc.vector.tensor_copy(s_sb[:], banks[0][:, 0:SLICE])
                for b in range(1, NB):
                    nc.vector.tensor_add(s_sb[:], s_sb[:], banks[b][:, 0:SLICE])

            # ---- epilogue: build W = [W_real | W_imag] (64, 128) ----
            with tc.tile_pool(name="vpsum", bufs=1, space="PSUM") as vpsum:
                v1 = vpsum.tile([64, 64], fp32, tag="v1")
                nc.tensor.matmul(v1[:], lhsT=c_sb[:, 0:64], rhs=s_sb[:, 0:64],
                                 start=True, stop=False)
                nc.tensor.matmul(v1[:], lhsT=c_sb[:, 64:128],
                                 rhs=s_sb[:, 64:128], start=False, stop=True)
                v2 = vpsum.tile([64, 64], fp32, tag="v2")
                nc.tensor.matmul(v2[:], lhsT=c_sb[:, 128:192],
                                 rhs=s_sb[:, 0:64], start=True, stop=False)
                nc.tensor.matmul(v2[:], lhsT=c_sb[:, 192:256],
                                 rhs=s_sb[:, 64:128], start=False, stop=True)

                v1s = small.tile([64, 64], fp32, tag="v1s")
                nc.vector.tensor_copy(v1s[:], v1[:])
                v2s = small.tile([64, 64], fp32, tag="v2s")
                nc.vector.tensor_copy(v2s[:], v2[:])

            # mr = Mr (dup-stacked), mp = -Mi (dup-stacked)
            mr = small.tile([64, 32], fp32, tag="mr")
            mp = small.tile([64, 32], fp32, tag="mp")
            nc.vector.tensor_sub(mr[0:32, :], v1s[0:32, 0:64:2], v2s[0:32, 1:64:2])
            nc.vector.tensor_sub(mr[32:64, :], v2s[32:64, 0:64:2], v1s[32:64, 1:64:2])
            nc.vector.tensor_add(mp[0:32, :], v1s[0:32, 1:64:2], v2s[0:32, 0:64:2])
            nc.vector.tensor_add(mp[32:64, :], v2s[32:64, 1:64:2], v1s[32:64, 0:64:2])

            wri = small.tile([64, 128], fp32, tag="wri")
            s_ = float(SCALE)
            # W_real = [[Mr, -Mi], [-Mi, -Mr]] * s
            nc.scalar.mul(wri[0:32, 0:32], mr[0:32, :], s_)
            nc.scalar.mul(wri[0:32, 32:64], mp[0:32, :], s_)
            nc.scalar.mul(wri[32:64, 0:32], mp[32:64, :], s_)
            nc.scalar.mul(wri[32:64, 32:64], mr[32:64, :], -s_)
            # W_imag = [[Mi, Mr], [Mr, -Mi]] * s
            nc.scalar.mul(wri[0:32, 64:96], mp[0:32, :], -s_)
            nc.scalar.mul(wri[0:32, 96:128], mr[0:32, :], s_)
            nc.scalar.mul(wri[32:64, 64:96], mr[32:64, :], s_)
            nc.scalar.mul(wri[32:64, 96:128], mp[32:64, :], s_)

            # fp16 W for the phase-3 matmuls; fp16 ones-selector for the
            # csi reduction (matmul operands must both be non-fp32)
            wh = small.tile([64, 128], fp16, tag="wh")
            nc.vector.tensor_copy(wh[:], wri[:])
            sel16 = small.tile([128, 2], fp16, tag="sel16")
            nc.vector.tensor_copy(sel16[:], c_sb[:, 256:258])

            # PE warm-keepers: cheap matmuls dependent on s_sb bridge the
            # epilogue gap so HAM doesn't re-throttle before phase 3
            with tc.tile_pool(name="wpsum", bufs=1, space="PSUM") as wpsum:
                warm_ps = wpsum.tile([64, 64], fp32, tag="warm")
                for w in range(10):
                    nc.tensor.matmul(warm_ps[:], lhsT=c_sb[:, 0:64],
                                     rhs=s_sb[:, 0:64], start=True, stop=True)

            # ---- phase 3: csi chunks over F, software-pipelined so the PE
            # order is T0 T1 c0 T2 c1 ... (csi_i never stalls on e_i) ----
            csi_sb = small.tile([2, F], fp32, tag="csi_sb")
            with (
                tc.tile_pool(name="tpsum", bufs=4, space="PSUM") as tpsum,
                tc.tile_pool(name="cpsum", bufs=2, space="PSUM") as cpsum,
            ):
                e_tiles = [None] * N_FCHUNK

                def issue_T(ci):
                    fs = slice(ci * FCHUNK, (ci + 1) * FCHUNK)
                    t_ps = tpsum.tile([128, FCHUNK], fp32, tag="t",
                                      name=f"t{ci}")
                    # T = W^T g, fp16 single pass
                    nc.tensor.matmul(t_ps[:], lhsT=wh[:], rhs=g2_sb[0:64, fs],
                                     start=True, stop=True)
                    e_sb = epool.tile([128, FCHUNK], fp16, tag="e",
                                      name=f"e{ci}")
                    nc.vector.tensor_mul(e_sb[:], g2_sb[:, fs], t_ps[:])
                    e_tiles[ci] = e_sb

                def issue_csi(ci):
                    fs = slice(ci * FCHUNK, (ci + 1) * FCHUNK)
                    c_ps = cpsum.tile([2, FCHUNK], fp32, tag="c",
                                      name=f"c{ci}")
                    nc.tensor.matmul(c_ps[:], lhsT=sel16[:],
                                     rhs=e_tiles[ci][:], start=True, stop=True)
                    nc.scalar.copy(csi_sb[:, fs], c_ps[:])

                issue_T(0)
                for ci in range(1, N_FCHUNK):
                    issue_T(ci)
                    issue_csi(ci - 1)
                issue_csi(N_FCHUNK - 1)

            nc.sync.dma_start(out_d[:], csi_sb[:])

    nc.compile()
    return nc


def _prep_shared(fbv):
    """gth (64, F) fp16 from complex fbv (F, R): rows = [Re ranks; Im ranks]."""
    fbv32 = np.ascontiguousarray(fbv).view(np.float32).reshape(F, 2 * R)
    gbt = np.ascontiguousarray(
        np.concatenate([fbv32[:, 0::2].T, fbv32[:, 1::2].T], axis=0))
    return gbt.astype(np.float16)


def _shard_h(arr, core):
    """Core's complex64 shard -> fp16 hi array (N_MACRO, 128, MACRO_COLS)."""
    sh = arr[core * DIR_PER_CORE:(core + 1) * DIR_PER_CORE]
    f32 = np.ascontiguousarray(sh).view(np.float32).ravel()
    return f32.astype(np.float16).reshape(N_MACRO, 128, MACRO_COLS)


def kernel(attenuation_vectors, radiation_vectors, frequency_basis_vectors):
    from concourse.bass_utils import run_bass_kernel_spmd

    if "nc" not in _NC_CACHE:
        _NC_CACHE["nc"] = build_nc()
    nc = _NC_CACHE["nc"]

    gth = _prep_shared(frequency_basis_vectors)
    consts = _build_consts()
    in_maps = []
    for c in range(N_CORES):
        in_maps.append({
            "rad_h": _shard_h(radiation_vectors, c),
            "att_h": _shard_h(attenuation_vectors, c),
            "gth": gth,
            "consts": consts,
        })

    res = run_bass_kernel_spmd(nc, in_maps, core_ids=list(range(N_CORES)))
    acc = np.zeros((2, F), np.float64)
    for r in res.results:
        acc += r["csi"]
    return (acc[0] + 1j * acc[1]).astype(np.complex64)


# revision 11
# speedup vs baseline: 1.3747x; 1.0733x over previous
"""Trainium2 Bass kernel for LowRankRayTracer.

csi[f] = (delta_t/D) * v_f^T M v_f,  M = conj(rad)^T conj(att)  (R=32, complex)
contracted over N = D*K = 524288 rows.

Strategy (8 cores):
  - Shard the N rows across cores (512 directions each). csi is linear in M,
    so each core computes its partial S = rad^T att (128x128 f32; complex
    pairs via the f32 view + 2-rows-per-partition packing), builds
    W = [W_real|W_imag], computes partial csi over ALL F=8192 subcarriers,
    and the host sums the 8 partial csi vectors.
  - Precision budget: harness gate is rel_err < 2e-2, fp16-quantized inputs
    give ~5e-4, so rad/att/g/W/e are all fp16 "hi" only (no lo-correction
    passes): half the HBM bytes and a quarter of the PE columns vs the
    hi/lo-exact version.
  - Every load is split into multiple dma_starts to spread bytes evenly over
    the 16 DMA queues (a single dma_start lands on one ~22 GB/s queue).
  - Matmuls accumulate round-robin into 4 bank-sized PSUM tiles (avoids
    same-bank RMW serialization); phase 3 is software-pipelined so the PE
    never waits on the vector engine between chunks.
"""

import numpy as np

D, K, R = 4096, 128, 32
F = 8192
N_CORES = 8
DIR_PER_CORE = D // N_CORES              # 512
N_MACRO = 8                              # macro tiles per tensor per core
MACRO_COLS = 4096                        # fp16 per partition per macro tile
SLICE = 128                              # matmul slice width (2 rows/partition)
SCALE = (200.0 / K) / D                  # delta_t / num_directions (exact binary)
FCHUNK = 512                             # phase-3 subcarriers per chunk
                                         # (matmul PSUM out max = 1 bank)
N_FCHUNK = F // FCHUNK                   # 16
NB = 4                                   # round-robin PSUM accumulator banks

_NC_CACHE = {}


def _build_consts():
    """(128, 258) f32: four (128,64) selection matrices + ones-selector cols."""
    c = np.zeros((128, 258), np.float32)
    EA = np.zeros((128, 32), np.float32)
    OA = np.zeros((128, 32), np.float32)
    EB = np.zeros((128, 32), np.float32)
    OB = np.zeros((128, 32), np.float32)
    for m in range(32):
        EA[2 * m, m] = 1.0
        OA[2 * m + 1, m] = 1.0
        EB[64 + 2 * m, m] = 1.0
        OB[64 + 2 * m + 1, m] = 1.0
    c[:, 0:32] = EA
    c[:, 32:64] = OA
    c[:, 64:96] = EB
    c[:, 96:128] = OB
    c[:, 128:160] = OA
    c[:, 160:192] = EA
    c[:, 192:224] = OB
    c[:, 224:256] = EB
    c[0:64, 256] = 1.0
    c[64:128, 257] = 1.0
    return c


def build_nc(n_macro=N_MACRO):
    import concourse.bacc as bacc
    import concourse.mybir as mybir
    import concourse.tile as tile

    fp32 = mybir.dt.float32
    fp16 = mybir.dt.float16
    nc = bacc.Bacc(trn_type="TRN2", target_bir_lowering=False, debug=False)

    rad_d = nc.dram_tensor("rad_h", [n_macro, 128, MACRO_COLS], fp16,
                           kind="ExternalInput").ap()
    att_d = nc.dram_tensor("att_h", [n_macro, 128, MACRO_COLS], fp16,
                           kind="ExternalInput").ap()
    gth_d = nc.dram_tensor("gth", [64, F], fp16, kind="ExternalInput").ap()
    cst_d = nc.dram_tensor("consts", [128, 258], fp32, kind="ExternalInput").ap()
    out_d = nc.dram_tensor("csi", [2, F], fp32, kind="ExternalOutput").ap()

    with tile.TileContext(nc) as tc:
        with (
            # bufs=8: all macros resident in SBUF so every bulk dma_start
            # issues immediately and the 16 queues stay 100% fed
            tc.tile_pool(name="io", bufs=8) as io_pool,
            tc.tile_pool(name="small", bufs=1) as small,
            tc.tile_pool(name="epool", bufs=16) as epool,
        ):
            # constants up front (tiny); g loads interleaved after early macros
            c_sb = small.tile([128, 258], fp32, tag="consts")
            nc.sync.dma_start(c_sb[:], cst_d[:])
            g2_sb = small.tile([128, F], fp16, tag="g2")

            # ---- main loop: S += rad^T att, fp16 hi-only ----
            s_sb = small.tile([128, 128], fp32, tag="s_sb")
            n_slices = MACRO_COLS // SLICE
            total = n_macro * n_slices
            with tc.tile_pool(name="spsum", bufs=1, space="PSUM") as spsum:
                banks = [spsum.tile([128, 512], fp32, tag=f"s{b}",
                                    name=f"sbank{b}")
                         for b in range(NB)]
                seen = [False] * NB
                idx = 0
                for i in range(n_macro):
                    rad = io_pool.tile([128, MACRO_COLS], fp16, tag="rad")
                    att = io_pool.tile([128, MACRO_COLS], fp16, tag="att")
                    # chunked loads: spread bytes over many DMA queues, and
                    # let the first matmuls start after ~0.5 MiB, not 2 MiB
                    nch = 4 if i == 0 else 2
                    cm = MACRO_COLS // nch
                    for q in range(nch):
                        qs = slice(q * cm, (q + 1) * cm)
                        nc.sync.dma_start(rad[:, qs], rad_d[i, :, qs])
                        nc.scalar.dma_start(att[:, qs], att_d[i, :, qs])
                    if i in (1, 2):
                        # duplicated g (fp16, 128 partitions): the [64, F]
                        # tensor read twice from HBM, 4 chunked dma_starts
                        # per half, interleaved mid-loop so they neither
                        # steal first-macro bandwidth nor gate phase 3
                        half = (0, 64) if i == 1 else (64, 128)
                        for q in range(4):
                            fs = slice(q * (F // 4), (q + 1) * (F // 4))
                            nc.sync.dma_start(
                                g2_sb[half[0]:half[1], fs], gth_d[:, fs])
                    for s in range(n_slices):
                        sl = slice(s * SLICE, (s + 1) * SLICE)
                        b = idx % NB
                        nc.tensor.matmul(
                            banks[b][:, 0:SLICE],
                            lhsT=rad[:, sl],
                            rhs=att[:, sl],
                            start=not seen[b],
                            stop=(idx >= total - NB),
                        )
                        seen[b] = True
                        idx += 1

                # S = sum of the 4 round-robin banks (DVE may read at most
                # one PSUM operand per instruction)
                nc.vector.tensor_copy(s_sb[:], banks[0][:, 0:SLICE])
                for b in range(1, NB):
                    nc.vector.tensor_add(s_sb[:], s_sb[:], banks[b][:, 0:SLICE])

            # PE warm stream: the Tensor engine only reaches its max p-state
            # after ~3us of gap-free execution, so bridge every PE stall in
            # the epilogue with back-to-back fp16 matmuls on data that is
            # already resident (g2). Phase 3 then runs at full clock.
            warmp_ctx = tc.tile_pool(name="warmp", bufs=1, space="PSUM")
            warmp = warmp_ctx.__enter__()
            warm_ps = warmp.tile([128, 512], fp32, tag="warm")

            def warm(n):
                for _ in range(n):
                    nc.tensor.matmul(warm_ps[:], lhsT=g2_sb[0:64, 0:SLICE],
                                     rhs=g2_sb[0:64, 0:FCHUNK],
                                     start=True, stop=True)

            warm(6)  # covers the DVE bank-sum latency

            # ---- epilogue: build W = [W_real | W_imag] (64, 128) ----
            with tc.tile_pool(name="vpsum", bufs=1, space="PSUM") as vpsum:
                v1 = vpsum.tile([64, 64], fp32, tag="v1")
                nc.tensor.matmul(v1[:], lhsT=c_sb[:, 0:64], rhs=s_sb[:, 0:64],
                                 start=True, stop=False)
                nc.tensor.matmul(v1[:], lhsT=c_sb[:, 64:128],
                                 rhs=s_sb[:, 64:128], start=False, stop=True)
                v2 = vpsum.tile([64, 64], fp32, tag="v2")
                nc.tensor.matmul(v2[:], lhsT=c_sb[:, 128:192],
                                 rhs=s_sb[:, 0:64], start=True, stop=False)
                nc.tensor.matmul(v2[:], lhsT=c_sb[:, 192:256],
                                 rhs=s_sb[:, 64:128], start=False, stop=True)

                v1s = small.tile([64, 64], fp32, tag="v1s")
                nc.vector.tensor_copy(v1s[:], v1[:])
                v2s = small.tile([64, 64], fp32, tag="v2s")
                nc.vector.tensor_copy(v2s[:], v2[:])

            # mr = Mr (dup-stacked), mp = -Mi (dup-stacked)
            mr = small.tile([64, 32], fp32, tag="mr")
            mp = small.tile([64, 32], fp32, tag="mp")
            nc.vector.tensor_sub(mr[0:32, :], v1s[0:32, 0:64:2], v2s[0:32, 1:64:2])
            nc.vector.tensor_sub(mr[32:64, :], v2s[32:64, 0:64:2], v1s[32:64, 1:64:2])
            nc.vector.tensor_add(mp[0:32, :], v1s[0:32, 1:64:2], v2s[0:32, 0:64:2])
            nc.vector.tensor_add(mp[32:64, :], v2s[32:64, 1:64:2], v1s[32:64, 0:64:2])

            wri = small.tile([64, 128], fp32, tag="wri")
            s_ = float(SCALE)
            # W_real = [[Mr, -Mi], [-Mi, -Mr]] * s
            nc.scalar.mul(wri[0:32, 0:32], mr[0:32, :], s_)
            nc.scalar.mul(wri[0:32, 32:64], mp[0:32, :], s_)
            nc.scalar.mul(wri[32:64, 0:32], mp[32:64, :], s_)
            nc.scalar.mul(wri[32:64, 32:64], mr[32:64, :], -s_)
            # W_imag = [[Mi, Mr], [Mr, -Mi]] * s
            nc.scalar.mul(wri[0:32, 64:96], mp[0:32, :], -s_)
            nc.scalar.mul(wri[0:32, 96:128], mr[0:32, :], s_)
            nc.scalar.mul(wri[32:64, 64:96], mr[32:64, :], s_)
            nc.scalar.mul(wri[32:64, 96:128], mp[32:64, :], s_)

            # fp16 W for the phase-3 matmuls; fp16 ones-selector for the
            # csi reduction (matmul operands must both be non-fp32)
            wh = small.tile([64, 128], fp16, tag="wh")
            nc.vector.tensor_copy(wh[:], wri[:])
            sel16 = small.tile([128, 2], fp16, tag="sel16")
            nc.vector.tensor_copy(sel16[:], c_sb[:, 256:258])

            warm(10)  # covers the extract/W-build/convert latency chain
            warmp_ctx.__exit__(None, None, None)

            # ---- phase 3: csi chunks over F. All T matmuls first (shared
            # stationary, no reloads), e-muls run on DVE in parallel, then
            # the csi matmuls stream back-to-back ----
            csi_sb = small.tile([2, F], fp32, tag="csi_sb")
            with (
                tc.tile_pool(name="tpsum", bufs=4, space="PSUM") as tpsum,
                tc.tile_pool(name="cpsum", bufs=2, space="PSUM") as cpsum,
            ):
                e_tiles = []
                for ci in range(N_FCHUNK):
                    fs = slice(ci * FCHUNK, (ci + 1) * FCHUNK)
                    t_ps = tpsum.tile([128, FCHUNK], fp32, tag="t",
                                      name=f"t{ci}")
                    # T = W^T g, fp16 single pass
                    nc.tensor.matmul(t_ps[:], lhsT=wh[:], rhs=g2_sb[0:64, fs],
                                     start=True, stop=True)
                    e_sb = epool.tile([128, FCHUNK], fp16, tag="e",
                                      name=f"e{ci}")
                    nc.vector.tensor_mul(e_sb[:], g2_sb[:, fs], t_ps[:])
                    e_tiles.append(e_sb)
                for ci in range(N_FCHUNK):
                    fs = slice(ci * FCHUNK, (ci + 1) * FCHUNK)
                    c_ps = cpsum.tile([2, FCHUNK], fp32, tag="c",
                                      name=f"c{ci}")
                    nc.tensor.matmul(c_ps[:], lhsT=sel16[:],
                                     rhs=e_tiles[ci][:], start=True, stop=True)
                    nc.scalar.copy(csi_sb[:, fs], c_ps[:])
                    # stream the output out per chunk instead of one final DMA
                    nc.sync.dma_start(out_d[:, fs], csi_sb[:, fs])

    nc.compile()
    return nc


def _prep_shared(fbv):
    """gth (64, F) fp16 from complex fbv (F, R): rows = [Re ranks; Im ranks]."""
    fbv32 = np.ascontiguousarray(fbv).view(np.float32).reshape(F, 2 * R)
    gbt = np.ascontiguousarray(
        np.concatenate([fbv32[:, 0::2].T, fbv32[:, 1::2].T], axis=0))
    return gbt.astype(np.float16)


def _shard_h(arr, core):
    """Core's complex64 shard -> fp16 hi array (N_MACRO, 128, MACRO_COLS)."""
    sh = arr[core * DIR_PER_CORE:(core + 1) * DIR_PER_CORE]
    f32 = np.ascontiguousarray(sh).view(np.float32).ravel()
    return f32.astype(np.float16).reshape(N_MACRO, 128, MACRO_COLS)


def kernel(attenuation_vectors, radiation_vectors, frequency_basis_vectors):
    from concourse.bass_utils import run_bass_kernel_spmd

    if "nc" not in _NC_CACHE:
        _NC_CACHE["nc"] = build_nc()
    nc = _NC_CACHE["nc"]

    gth = _prep_shared(frequency_basis_vectors)
    consts = _build_consts()
    in_maps = []
    for c in range(N_CORES):
        in_maps.append({
            "rad_h": _shard_h(radiation_vectors, c),
            "att_h": _shard_h(attenuation_vectors, c),
            "gth": gth,
            "consts": consts,
        })

    res = run_bass_kernel_spmd(nc, in_maps, core_ids=list(range(N_CORES)))
    acc = np.zeros((2, F), np.float64)
    for r in res.results:
        acc += r["csi"]
    return (acc[0] + 1j * acc[1]).astype(np.complex64)


# revision 15
# speedup vs baseline: 1.4027x; 1.0203x over previous
"""Trainium2 Bass kernel for LowRankRayTracer.

csi[f] = (delta_t/D) * v_f^T M v_f,  M = conj(rad)^T conj(att)  (R=32, complex)
contracted over N = D*K = 524288 rows.

Strategy (8 cores):
  - Shard the N rows across cores (512 directions each). csi is linear in M,
    so each core computes its partial S = rad^T att (128x128 f32; complex
    pairs via the f32 view + 2-rows-per-partition packing), builds
    W = [W_real|W_imag] in fp16, computes partial csi over ALL F=8192
    subcarriers, and the host sums the 8 partial csi vectors.
  - Precision budget: harness gate is rel_err < 2e-2, fp16-quantized inputs
    give ~5e-4, so rad/att/g/W/e are all fp16 "hi" only (no lo-correction
    passes): half the HBM bytes and a quarter of the PE columns vs the
    hi/lo-exact version.
  - Every load is split into multiple dma_starts to spread bytes evenly over
    the 16 DMA queues (one dma_start lands on one ~20 GB/s queue); io bufs=8
    keeps all of them in flight so the queues never starve.
  - The PE only reaches max clock (2.4 GHz) after ~3us gap-free execution and
    drops to 1.2 GHz after any stall, so a warm-matmul stream bridges every
    PE bubble between the main loop and phase 3.
  - Epilogue W-build runs as add/subs of pre-scaled S selections, split
    across the Vector and GpSimd engines, writing the fp16 W directly.
  - Matmuls accumulate round-robin into 4 bank-sized PSUM tiles (avoids
    same-bank RMW serialization); phase-3 csi results DMA straight from PSUM.
"""

import numpy as np

D, K, R = 4096, 128, 32
F = 8192
N_CORES = 8
DIR_PER_CORE = D // N_CORES              # 512
N_MACRO = 8                              # macro tiles per tensor per core
MACRO_COLS = 4096                        # fp16 per partition per macro tile
SLICE = 128                              # matmul slice width (2 rows/partition)
SCALE = (200.0 / K) / D                  # delta_t / num_directions (exact binary)
FCHUNK = 512                             # phase-3 subcarriers per chunk
                                         # (matmul PSUM out max = 1 bank)
N_FCHUNK = F // FCHUNK                   # 16
NB = 4                                   # round-robin PSUM accumulator banks

_NC_CACHE = {}


def _build_consts():
    """(128, 258) f32: four (128,64) selection matrices + ones-selector cols."""
    c = np.zeros((128, 258), np.float32)
    EA = np.zeros((128, 32), np.float32)
    OA = np.zeros((128, 32), np.float32)
    EB = np.zeros((128, 32), np.float32)
    OB = np.zeros((128, 32), np.float32)
    for m in range(32):
        EA[2 * m, m] = 1.0
        OA[2 * m + 1, m] = 1.0
        EB[64 + 2 * m, m] = 1.0
        OB[64 + 2 * m + 1, m] = 1.0
    c[:, 0:32] = EA
    c[:, 32:64] = OA
    c[:, 64:96] = EB
    c[:, 96:128] = OB
    c[:, 128:160] = OA
    c[:, 160:192] = EA
    c[:, 192:224] = OB
    c[:, 224:256] = EB
    c[0:64, 256] = 1.0
    c[64:128, 257] = 1.0
    return c


def build_nc(n_macro=N_MACRO):
    import concourse.bacc as bacc
    import concourse.mybir as mybir
    import concourse.tile as tile

    fp32 = mybir.dt.float32
    fp16 = mybir.dt.float16
    nc = bacc.Bacc(trn_type="TRN2", target_bir_lowering=False, debug=False)

    rad_d = nc.dram_tensor("rad_h", [n_macro, 128, MACRO_COLS], fp16,
                           kind="ExternalInput").ap()
    att_d = nc.dram_tensor("att_h", [n_macro, 128, MACRO_COLS], fp16,
                           kind="ExternalInput").ap()
    gth_d = nc.dram_tensor("gth", [64, F], fp16, kind="ExternalInput").ap()
    cst_d = nc.dram_tensor("consts", [128, 258], fp32, kind="ExternalInput").ap()
    out_d = nc.dram_tensor("csi", [2, F], fp32, kind="ExternalOutput").ap()

    with tile.TileContext(nc) as tc:
        with (
            # bufs=8: all macros resident in SBUF so every bulk dma_start
            # issues immediately and the 16 queues stay fed
            tc.tile_pool(name="io", bufs=8) as io_pool,
            tc.tile_pool(name="small", bufs=1) as small,
            tc.tile_pool(name="epool", bufs=16) as epool,
            # warm-matmul PSUM scratch gets its own bank up front so warm
            # matmuls never wait on bank reuse (a stall would drop p-state)
            tc.tile_pool(name="warmp", bufs=1, space="PSUM") as warmp,
        ):
            c_sb = small.tile([128, 258], fp32, tag="consts")
            nc.sync.dma_start(c_sb[:], cst_d[:])
            g2_sb = small.tile([128, F], fp16, tag="g2")
            # fp16 ones-selector for the csi reduction (built early, off the
            # critical engines; matmul operands must both be non-fp32)
            sel16 = small.tile([128, 2], fp16, tag="sel16")
            nc.gpsimd.tensor_copy(sel16[:], c_sb[:, 256:258])

            warm_ps = warmp.tile([128, FCHUNK], fp32, tag="warm")

            def warm(n):
                for _ in range(n):
                    nc.tensor.matmul(warm_ps[:], lhsT=g2_sb[0:64, 0:SLICE],
                                     rhs=g2_sb[0:64, 0:FCHUNK],
                                     start=True, stop=True)

            # ---- main loop: S += rad^T att, fp16 hi-only ----
            s_sb = small.tile([128, 128], fp32, tag="s_sb")
            n_slices = MACRO_COLS // SLICE
            total = n_macro * n_slices
            with tc.tile_pool(name="spsum", bufs=1, space="PSUM") as spsum:
                banks = [spsum.tile([128, 512], fp32, tag=f"s{b}",
                                    name=f"sbank{b}")
                         for b in range(NB)]
                seen = [False] * NB
                idx = 0
                for i in range(n_macro):
                    rad = io_pool.tile([128, MACRO_COLS], fp16, tag="rad")
                    att = io_pool.tile([128, MACRO_COLS], fp16, tag="att")
                    # chunked loads: spread bytes over many DMA queues, and
                    # let the first matmuls start after ~0.5 MiB, not 2 MiB
                    nch = 4 if i == 0 else 2
                    cm = MACRO_COLS // nch
                    for q in range(nch):
                        qs = slice(q * cm, (q + 1) * cm)
                        nc.sync.dma_start(rad[:, qs], rad_d[i, :, qs])
                        nc.scalar.dma_start(att[:, qs], att_d[i, :, qs])
                    if i in (1, 2):
                        # duplicated g (fp16, 128 partitions): the [64, F]
                        # tensor read twice from HBM, 4 chunked dma_starts
                        # per half, interleaved mid-loop so they neither
                        # steal first-macro bandwidth nor gate phase 3
                        half = (0, 64) if i == 1 else (64, 128)
                        for q in range(4):
                            fs = slice(q * (F // 4), (q + 1) * (F // 4))
                            nc.sync.dma_start(
                                g2_sb[half[0]:half[1], fs], gth_d[:, fs])
                    for s in range(n_slices):
                        sl = slice(s * SLICE, (s + 1) * SLICE)
                        b = idx % NB
                        nc.tensor.matmul(
                            banks[b][:, 0:SLICE],
                            lhsT=rad[:, sl],
                            rhs=att[:, sl],
                            start=not seen[b],
                            stop=(idx >= total - NB),
                        )
                        seen[b] = True
                        idx += 1

                # keep the PE streaming while the bank-sum runs
                warm(8)

                # S = sum of the 4 round-robin banks (Vector only: GpSimd
                # cannot access PSUM, and DVE reads max one PSUM operand)
                nc.vector.tensor_copy(s_sb[:], banks[0][:, 0:SLICE])
                for b in range(1, NB):
                    nc.vector.tensor_add(s_sb[:], s_sb[:], banks[b][:, 0:SLICE])

            # ---- epilogue: W = [W_real | W_imag] (64, 128) fp16, built as
            # add/subs of the pre-scaled selection matmul outputs ----
            with tc.tile_pool(name="vpsum", bufs=1, space="PSUM") as vpsum:
                v1 = vpsum.tile([64, 64], fp32, tag="v1")
                nc.tensor.matmul(v1[:], lhsT=c_sb[:, 0:64], rhs=s_sb[:, 0:64],
                                 start=True, stop=False)
                nc.tensor.matmul(v1[:], lhsT=c_sb[:, 64:128],
                                 rhs=s_sb[:, 64:128], start=False, stop=True)
                v2 = vpsum.tile([64, 64], fp32, tag="v2")
                nc.tensor.matmul(v2[:], lhsT=c_sb[:, 128:192],
                                 rhs=s_sb[:, 0:64], start=True, stop=False)
                nc.tensor.matmul(v2[:], lhsT=c_sb[:, 192:256],
                                 rhs=s_sb[:, 64:128], start=False, stop=True)

                # cover the whole DVE/Pool epilogue chain below
                warm(14)

                s_ = float(SCALE)
                a_sb = small.tile([64, 64], fp32, tag="a_sb")   # v1 * s
                b_sb = small.tile([64, 64], fp32, tag="b_sb")   # v2 * -s
                c2_sb = small.tile([64, 64], fp32, tag="c2_sb")  # v2 * s
                nc.vector.tensor_scalar_mul(a_sb[:], v1[:], s_)
                nc.vector.tensor_scalar_mul(b_sb[:], v2[:], -s_)
                # GpSimd cannot read PSUM; derive v2*s = -b in SBUF
                nc.gpsimd.tensor_scalar_mul(c2_sb[:], b_sb[:], -1.0)

            # W quadrants in fp16 directly. With a = v1*s, b = -v2*s,
            # c2 = v2*s and the dup-stacked row ranges:
            #   rows 0:32 : Mr*s = a_e + b_o, -Mi*s = a_o - b_e, Mi*s = b_e - a_o
            #   rows 32:64: -Mi*s = a_e - b_o, -Mr*s = b_e + a_o, Mr*s = c2_e - a_o
            wh = small.tile([64, 128], fp16, tag="wh")
            r1, r2 = slice(0, 32), slice(32, 64)
            ev, od = slice(0, 64, 2), slice(1, 64, 2)
            # W_real = [[Mr, -Mi], [-Mi, -Mr]] * s
            nc.vector.tensor_add(wh[r1, 0:32], a_sb[r1, ev], b_sb[r1, od])
            nc.vector.tensor_sub(wh[r1, 32:64], a_sb[r1, od], b_sb[r1, ev])
            nc.gpsimd.tensor_sub(wh[r2, 0:32], a_sb[r2, ev], b_sb[r2, od])
            nc.gpsimd.tensor_add(wh[r2, 32:64], b_sb[r2, ev], a_sb[r2, od])
            # W_imag = [[Mi, Mr], [Mr, -Mi]] * s
            nc.vector.tensor_sub(wh[r1, 64:96], b_sb[r1, ev], a_sb[r1, od])
            nc.vector.tensor_add(wh[r1, 96:128], a_sb[r1, ev], b_sb[r1, od])
            nc.gpsimd.tensor_sub(wh[r2, 64:96], c2_sb[r2, ev], a_sb[r2, od])
            nc.gpsimd.tensor_sub(wh[r2, 96:128], a_sb[r2, ev], b_sb[r2, od])

            # ---- phase 3: csi chunks over F. All T matmuls first (shared
            # stationary), e-muls on DVE in parallel, then the csi matmuls
            # stream back-to-back and their results DMA straight from PSUM ----
            with (
                tc.tile_pool(name="tpsum", bufs=4, space="PSUM") as tpsum,
                tc.tile_pool(name="cpsum", bufs=3, space="PSUM") as cpsum,
            ):
                e_tiles = []
                for ci in range(N_FCHUNK):
                    fs = slice(ci * FCHUNK, (ci + 1) * FCHUNK)
                    t_ps = tpsum.tile([128, FCHUNK], fp32, tag="t",
                                      name=f"t{ci}")
                    # T = W^T g, fp16 single pass
                    nc.tensor.matmul(t_ps[:], lhsT=wh[:], rhs=g2_sb[0:64, fs],
                                     start=True, stop=True)
                    e_sb = epool.tile([128, FCHUNK], fp16, tag="e",
                                      name=f"e{ci}")
                    nc.vector.tensor_mul(e_sb[:], g2_sb[:, fs], t_ps[:])
                    e_tiles.append(e_sb)
                csi_sb = small.tile([2, F], fp32, tag="csi_sb")
                for ci in range(N_FCHUNK):
                    fs = slice(ci * FCHUNK, (ci + 1) * FCHUNK)
                    c_ps = cpsum.tile([2, FCHUNK], fp32, tag="c",
                                      name=f"c{ci}")
                    nc.tensor.matmul(c_ps[:], lhsT=sel16[:],
                                     rhs=e_tiles[ci][:], start=True, stop=True)
                    # DVE (idle here) drains PSUM; DMA cannot read PSUM
                    nc.vector.tensor_copy(csi_sb[:, fs], c_ps[:])
                    nc.sync.dma_start(out_d[:, fs], csi_sb[:, fs])

    nc.compile()
    return nc


def _prep_shared(fbv):
    """gth (64, F) fp16 from complex fbv (F, R): rows = [Re ranks; Im ranks]."""
    fbv32 = np.ascontiguousarray(fbv).view(np.float32).reshape(F, 2 * R)
    gbt = np.ascontiguousarray(
        np.concatenate([fbv32[:, 0::2].T, fbv32[:, 1::2].T], axis=0))
    return gbt.astype(np.float16)


def _shard_h(arr, core):
    """Core's complex64 shard -> fp16 hi array (N_MACRO, 128, MACRO_COLS)."""
    sh = arr[core * DIR_PER_CORE:(core + 1) * DIR_PER_CORE]
    f32 = np.ascontiguousarray(sh).view(np.float32).ravel()
    return f32.astype(np.float16).reshape(N_MACRO, 128, MACRO_COLS)


def kernel(attenuation_vectors, radiation_vectors, frequency_basis_vectors):
    from concourse.bass_utils import run_bass_kernel_spmd

    if "nc" not in _NC_CACHE:
        _NC_CACHE["nc"] = build_nc()
    nc = _NC_CACHE["nc"]

    gth = _prep_shared(frequency_basis_vectors)
    consts = _build_consts()
    in_maps = []
    for c in range(N_CORES):
        in_maps.append({
            "rad_h": _shard_h(radiation_vectors, c),
            "att_h": _shard_h(attenuation_vectors, c),
            "gth": gth,
            "consts": consts,
        })

    res = run_bass_kernel_spmd(nc, in_maps, core_ids=list(range(N_CORES)))
    acc = np.zeros((2, F), np.float64)
    for r in res.results:
        acc += r["csi"]
    return (acc[0] + 1j * acc[1]).astype(np.complex64)
